# revision 1
# baseline (speedup 1.0000x reference)
"""AttentionBlock (GroupNorm -> 1x1 qkv -> 4-head attention -> 1x1 proj -> residual)
on 8 trn2 NeuronCores, data-parallel over the batch dim (B=8, one element/core).

Layout per core: channel-major [C=512, N=1024] as 4 SBUF tiles of [128, 1024].
V is computed spatial-major directly from the qkv matmul so attention needs no
transposes:
  ST[j,i] = sum_d k[d,j] q[d,i]   (K-tile stationary)
  p~T[j,i] = exp(scale*ST)        (ScalarE, PSUM->SBUF)
  rowsum   = gpsimd partition_all_reduce over the bf16 pairwise-sum tree
  PV[d,i] = sum_j v_sp[j,d] p~T[j,i]   -> channel-major attention output
  out = PV * (1/rowsum)                (softmax divide deferred past PV)
v-bias is folded into proj bias on the host (softmax rows sum to 1).

Schedule notes:
- gpsimd SWDGE stripes DMA engines (~200 GB/s) vs ~15-20 GB/s per HWDGE
  queue, so the xb tiles + the q0/k0 weight columns stream on SWDGE in
  dependency order; head 0's qkv matmuls ride each GroupNorm tile (wave A)
  so the first exp fires as early as possible.
- GroupNorm broadcasts raw group sums through the PE (two back-to-back tiny
  matmuls) and does all per-channel math at [128,1] on DVE, minimizing
  cross-engine hops; rstd uses sqrt(eps - (mean^2 - msq)) via scale=-1.
- The softmax divide uses partition_all_reduce + full-width reciprocal, so
  no PSUM rowsum tiles exist and PV of head h+1 never waits on the divide
  chain of head h (the 2-deep ppv ring only couples h+1 to h-1).
- residual is added from the bf16 x copy and the output is stored bf16
  (error budget is 2e-2; this measures ~6e-3).
"""

import numpy as np

B, C, H, W = 8, 512, 32, 32
N = H * W  # 1024
NUM_HEADS = 4
HEAD_DIM = C // NUM_HEADS  # 128
NUM_GROUPS = 32
GROUP_CH = C // NUM_GROUPS  # 16
EPS = 1e-5
NT = C // 128  # 4 channel tiles
NO_QK = 8  # q,k output tiles (1024 channels)
SCALE = 1.0 / float(np.sqrt(HEAD_DIM))
N_CORES = 8


def build_bass():
    import concourse.bacc as bacc
    import concourse.tile as tile
    from concourse import mybir
    from concourse import bass_isa

    f32 = mybir.dt.float32
    bf16 = mybir.dt.bfloat16
    Act = mybir.ActivationFunctionType
    Alu = mybir.AluOpType
    Ax = mybir.AxisListType

    nc = bacc.Bacc("TRN2", target_bir_lowering=False, debug=False,
                   num_devices=N_CORES)

    d_xb = nc.declare_dram_parameter("xb", [C, N], bf16, isOutput=False)
    d_wqkw = nc.declare_dram_parameter("qkw_wt", [C, 256], bf16,
                                       isOutput=False)
    d_wrest = nc.declare_dram_parameter("qkrest_wt", [C, 768], bf16,
                                        isOutput=False)
    d_wv = nc.declare_dram_parameter("v_wt", [C, C], bf16, isOutput=False)
    d_pwt = nc.declare_dram_parameter("proj_wt", [C, C], bf16, isOutput=False)
    d_cv = nc.declare_dram_parameter("cvec", [128, 28], f32, isOutput=False)
    d_ss = nc.declare_dram_parameter("selsel", [128, 128], f32,
                                    isOutput=False)
    d_ones = nc.declare_dram_parameter("ones", [128, 1], bf16, isOutput=False)
    d_out = nc.declare_dram_parameter("out", [C, N], bf16, isOutput=True)

    with tile.TileContext(nc) as tc:
        with (
            tc.tile_pool(name="persist", bufs=1) as pp,
            # 32 bufs = one slot per exp tile for the whole kernel: the pt
            # ring can never gate a late exp on PV consumption of an old tile
            tc.tile_pool(name="pt", bufs=32) as p_pt,
            tc.tile_pool(name="rsum", bufs=5) as p_rs,
            # one slot per output tile: with 2 bufs the wave-2 stores were
            # gated on wave-1's output DMA reads draining, at the fully
            # exposed end of the kernel
            tc.tile_pool(name="outp", bufs=4) as p_out,
            tc.tile_pool(name="small", bufs=1) as ps,
            tc.tile_pool(name="psum", bufs=2, space="PSUM") as pm,
        ):
            warm = ps.tile([128, 512], bf16, tag="warm", name="warm")
            nc.vector.memset(warm[:], 0.5)
            epsv = ps.tile([128, 1], f32, tag="epsv", name="epsv")
            nc.vector.memset(epsv[:], EPS)
            # pre-load the sqrt ACT table during the DMA dead zone so the
            # first GroupNorm rstd doesn't eat the 1.3us load
            dums = ps.tile([128, 1], f32, tag="dums", name="dums")
            nc.scalar.activation(dums[:], epsv[:, 0:1], Act.Sqrt)

            cvec = ps.tile([128, 28], f32, tag="cvec", name="cvec")
            gam, bet, bqk, beff = (cvec[:, 0:4], cvec[:, 4:8],
                                   cvec[:, 8:16], cvec[:, 16:20])
            sel = cvec[:, 20:28]
            selsel = ps.tile([128, 128], f32, tag="selsel", name="selsel")
            ones_r = ps.tile([128, 1], bf16, tag="ones_r", name="ones_r")
            nc.scalar.dma_start(ones_r[:], d_ones[:, :])

            # ---- bulk loads. SWDGE carries the GN->ST critical path in
            # dependency order (xb tiles paired with just the q0/k0 weight
            # columns); the rest of the qk weights, then v weights, follow
            # on the same queue timed to when the consuming matmuls start.
            # The two slow HWDGE queues take xb3 (split), proj weights and
            # the outputs.
            xbs, hs, wwaves, wrests, wvs, pwts = [], [], [], [], [], []
            for t in range(NT):
                xb_t = pp.tile([128, N], bf16, tag=f"xb{t}", name=f"xb{t}")
                xbs.append(xb_t)
            for t in range(NT):
                ww_t = pp.tile([128, 256], bf16, tag=f"ww{t}", name=f"ww{t}")
                wwaves.append(ww_t)
            for t in range(NT):
                wr_t = pp.tile([128, 768], bf16, tag=f"wr{t}", name=f"wr{t}")
                wrests.append(wr_t)
            for t in range(NT):
                wv_t = pp.tile([128, C], bf16, tag=f"wv{t}", name=f"wv{t}")
                wvs.append(wv_t)
            for t in range(2):
                nc.gpsimd.dma_start(xbs[t][:], d_xb[t * 128:(t + 1) * 128, :])
                nc.gpsimd.dma_start(wwaves[t][:],
                                    d_wqkw[t * 128:(t + 1) * 128, :])
            nc.gpsimd.dma_start(xbs[2][:], d_xb[256:384, :])
            nc.gpsimd.dma_start(xbs[3][:], d_xb[384:512, :])
            nc.sync.dma_start(selsel[:], d_ss[:, :])
            nc.sync.dma_start(cvec[:], d_cv[:, :])
            nc.sync.dma_start(wwaves[2][:], d_wqkw[256:384, :])
            nc.scalar.dma_start(wwaves[3][:], d_wqkw[384:512, :])
            for t in range(NT):
                nc.gpsimd.dma_start(wrests[t][:],
                                    d_wrest[t * 128:(t + 1) * 128, :])
            for t in range(NT):
                nc.gpsimd.dma_start(wvs[t][:], d_wv[t * 128:(t + 1) * 128, :])
            for t in range(NT):
                pwt_t = pp.tile([128, C], bf16, tag=f"pwt{t}", name=f"pwt{t}")
                eng = nc.sync if t < 2 else nc.scalar
                eng.dma_start(pwt_t[:], d_pwt[t * 128:(t + 1) * 128, :])
                pwts.append(pwt_t)

            # PE warm-up: junk matmul chain (never read) holds the PE p-state
            # up while the first xb tiles stream in.
            junk = pm.tile([128, N], f32, tag="acc", name="junk")

            def junk_mm(n, first=False, last=False):
                for j in range(n):
                    nc.tensor.matmul(junk[0:128, 0:512], warm[:, 0:128],
                                     warm[:, 0:512],
                                     start=(first and j == 0),
                                     stop=(last and j == n - 1),
                                     skip_group_check=True)

            junk_mm(6, first=True, last=True)

            # q0/k0 accumulate across GN tiles (wave A)
            pqq = pm.tile([128, N], f32, tag="acc", name="pqq")
            pqk = pm.tile([128, N], f32, tag="acc", name="pqk")

            # ---- group norm per-tile (groups never cross 128-ch tiles),
            # with the head-0 qkv matmuls riding along as h tiles appear.
            for t in range(NT):
                h_t = pp.tile([128, N], bf16, tag=f"h{t}", name=f"h{t}")
                hs.append(h_t)
            msrs = []
            for t in range(NT):
                st_t = ps.tile([128, 2], f32, tag=f"st{t}", name=f"st{t}")
                nc.vector.reduce_sum(st_t[:, 0:1], xbs[t][:], axis=Ax.X)
                nc.scalar.activation(hs[t][:], xbs[t][:], Act.Square,
                                     accum_out=st_t[:, 1:2])
                # one fused matmul broadcasts the group (sum, sumsq)/n
                # to channels: selsel = sel @ sel.T / n (block-diagonal,
                # host-precomputed), so no PSUM->SBUF hop between the group
                # reduce and the broadcast.
                pse = pm.tile([128, N], f32, tag="ps", name=f"pse{t}")
                nc.tensor.matmul(pse[:, 0:2], selsel[:], st_t[:, 0:2],
                                 start=True, stop=True)
                msr = ps.tile([128, 8], f32, tag=f"msr{t}", name=f"msr{t}")
                msrs.append(msr)
                nc.vector.tensor_copy(msr[:, 6:8], pse[:, 0:2])
                # negvar = mean^2 - msq ; rstd = 1/sqrt(eps - negvar)
                nc.vector.scalar_tensor_tensor(msr[:, 0:1], msr[:, 6:7],
                                               msr[:, 6:7], msr[:, 7:8],
                                               op0=Alu.mult, op1=Alu.subtract)
                nc.scalar.activation(msr[:, 1:2], msr[:, 0:1], Act.Sqrt,
                                     bias=epsv[:, 0:1], scale=-1.0)
                nc.vector.reciprocal(msr[:, 2:3], msr[:, 1:2])
                nc.vector.tensor_mul(msr[:, 3:4], gam[:, t:t + 1],
                                     msr[:, 2:3])
                nc.vector.tensor_mul(msr[:, 4:5], msr[:, 6:7], msr[:, 3:4])
                nc.vector.tensor_sub(msr[:, 5:6], bet[:, t:t + 1],
                                     msr[:, 4:5])
                if t == 0:
                    nc.scalar.activation(hs[t][:], xbs[t][:], Act.Identity,
                                         bias=msr[:, 5:6], scale=msr[:, 3:4])
                else:
                    nc.vector.tensor_scalar(hs[t][:], xbs[t][:],
                                            msr[:, 3:4], msr[:, 5:6],
                                            op0=Alu.mult, op1=Alu.add)
                # wave A: q0 and k0 ride the fresh h tile
                for pq, wc in ((pqq, 0), (pqk, 1)):
                    for half in range(2):
                        nc.tensor.matmul(
                            pq[:, half * 512:(half + 1) * 512],
                            wwaves[t][:, wc * 128:(wc + 1) * 128],
                            hs[t][:, half * 512:(half + 1) * 512],
                            start=(t == 0), stop=(t == NT - 1))

            # dummy exp reading t3's sqrt output: forces the exp-table load
            # to happen right after the last GN sqrt, off the critical path
            dum = ps.tile([128, 1], f32, tag="dum", name="dum")
            nc.scalar.activation(dum[:], msrs[3][:, 1:2], Act.Exp)

            qks = [None] * NO_QK
            vs = [None] * NO_QK
            all_pts = [[] for _ in range(NUM_HEADS)]
            attns = [None] * NUM_HEADS

            def emit_bias(ot, pq):
                # q biases on DVE, k biases on ACT: the DVE queue (trees,
                # casts) otherwise delays the k biases and starves the
                # downstream ST matmuls that feed the exp stream.
                qk_t = pp.tile([128, N], bf16, tag=f"qk{ot}", name=f"qk{ot}")
                if ot == NUM_HEADS:
                    nc.scalar.activation(qk_t[:], pq[:], Act.Identity,
                                         bias=bqk[:, ot:ot + 1])
                else:
                    nc.vector.tensor_scalar_add(qk_t[:], pq[:],
                                                bqk[:, ot:ot + 1])
                qks[ot] = qk_t

            def emit_qkv(ot):
                wc = (ot - 1) * 128 if ot < NUM_HEADS else 384 + (ot - 5) * 128
                pq = pm.tile([128, N], f32, tag="acc", name=f"pq{ot}")
                for t in range(NT):
                    for half in range(2):
                        nc.tensor.matmul(
                            pq[:, half * 512:(half + 1) * 512],
                            wrests[t][:, wc:wc + 128],
                            hs[t][:, half * 512:(half + 1) * 512],
                            start=(t == 0), stop=(t == NT - 1))
                emit_bias(ot, pq)

            def emit_v(nt):
                pv_ = pm.tile([128, N], f32, tag="acc", name=f"pvv{nt}")
                for t in range(NT):
                    nc.tensor.matmul(
                        pv_[:, 0:512],
                        hs[t][:, nt * 128:(nt + 1) * 128],
                        wvs[t][:, 0:512],
                        start=(t == 0), stop=(t == NT - 1))
                v_t = pp.tile([128, 512], bf16, tag=f"v{nt}", name=f"v{nt}")
                nc.vector.tensor_copy(v_t[:], pv_[:, 0:512])
                vs[nt] = v_t

            def emit_st(h, jts=None):
                qT = qks[h]
                kT = qks[NUM_HEADS + h]
                if jts is None:
                    jts = range(NO_QK)
                pts = all_pts[h]
                for jt in jts:
                    pst = pm.tile([128, N], f32, tag="ps", name=f"pst{h}_{jt}")
                    for half in range(2):
                        nc.tensor.matmul(
                            pst[:, half * 512:(half + 1) * 512],
                            kT[:, jt * 128:(jt + 1) * 128],
                            qT[:, half * 512:(half + 1) * 512],
                            start=True, stop=True)
                    pt_jt = p_pt.tile([128, N], bf16, tag="pt",
                                      name=f"pt{h}_{jt}")
                    nc.scalar.activation(pt_jt[:], pst[:], Act.Exp, scale=SCALE)
                    pts.append(pt_jt)
                all_pts[h] = pts

            def emit_tree(h):
                pts = all_pts[h]
                # pairwise row-sum tree over pt[0..6], all in bf16 (2x DVE
                # mode); pt[7] is folded in by a second accumulating
                # ones-matmul so the divide chain starts one exp earlier.
                # The two leading level-1 adds of the first two heads ride
                # the otherwise-idle GpSimd engine; later heads stay on DVE
                # so GpSimd is free for the broadcast chain at the tail.
                l1 = nc.gpsimd if h < 2 else nc.vector

                def add(eng, out, a, b):
                    # last head: per-half adds so the half-0 chain down to
                    # the divide/attn/proj runs at half latency per level
                    # (only its tree is on the exposed tail; mid-kernel the
                    # extra op overhead isn't worth it)
                    if h == NUM_HEADS - 1:
                        for hf in range(2):
                            sl = slice(hf * 512, (hf + 1) * 512)
                            eng.tensor_add(out[:, sl], a[:, sl], b[:, sl])
                    else:
                        eng.tensor_add(out[:], a[:], b[:])

                u01 = p_rs.tile([128, N], bf16, tag="rs1", name=f"u01_{h}")
                add(l1, u01, pts[0], pts[1])
                u23 = p_rs.tile([128, N], bf16, tag="rs1", name=f"u23_{h}")
                add(l1, u23, pts[2], pts[3])
                u45 = p_rs.tile([128, N], bf16, tag="rs1", name=f"u45_{h}")
                add(nc.vector, u45, pts[4], pts[5])
                u67 = p_rs.tile([128, N], bf16, tag="rs1", name=f"u67_{h}")
                add(nc.vector, u67, pts[6], pts[7])
                u0123 = p_rs.tile([128, N], bf16, tag="rs2", name=f"u0123_{h}")
                add(nc.vector, u0123, u01, u23)
                u4567 = p_rs.tile([128, N], bf16, tag="rs2", name=f"u4567_{h}")
                add(nc.vector, u4567, u45, u67)
                uallb = p_rs.tile([128, N], bf16, tag="rs2", name=f"uallb_{h}")
                add(nc.vector, uallb, u0123, u4567)
                return uallb, None

            def emit_tree_d7(h):
                # last head: leave pt[7] out of the tree (second accumulating
                # ones-matmul folds it in) so the divide chain starts one exp
                # earlier on the critical tail
                pts = all_pts[h]
                u01 = p_rs.tile([128, N], bf16, tag="rs1", name=f"u01_{h}")
                nc.vector.tensor_add(u01[:], pts[0][:], pts[1][:])
                u23 = p_rs.tile([128, N], bf16, tag="rs1", name=f"u23_{h}")
                nc.vector.tensor_add(u23[:], pts[2][:], pts[3][:])
                u45 = p_rs.tile([128, N], bf16, tag="rs1", name=f"u45_{h}")
                nc.vector.tensor_add(u45[:], pts[4][:], pts[5][:])
                u0123 = p_rs.tile([128, N], bf16, tag="rs2", name=f"u0123_{h}")
                nc.vector.tensor_add(u0123[:], u01[:], u23[:])
                u456 = p_rs.tile([128, N], bf16, tag="rs1", name=f"u456_{h}")
                nc.vector.tensor_add(u456[:], u45[:], pts[6][:])
                uallb = p_rs.tile([128, N], bf16, tag="rs2", name=f"uallb_{h}")
                nc.vector.tensor_add(uallb[:], u0123[:], u456[:])
                return uallb, pts[7]

            def emit_pv_st(hp, hn, uallb, upt7=None):
                # interleave PV of head hp with ST/exp of head hn at jt
                # granularity: ACT's exp stream stays fed while PE does PV.
                # The last head's PSUM tiles ride the "ps" ring (free once
                # the final STs drain) instead of "acc", so PV3 does not
                # wait for head 2's divide chain (the acc ring would gate
                # ppv3 on attn2's multiply).
                ptag = "ps" if hp == NUM_HEADS - 1 else "acc"
                pts = all_pts[hp] if hp is not None else None
                ppv = None
                if hp is not None:
                    ppv = pm.tile([128, N], f32, tag=ptag, name=f"ppv{hp}")
                prs = None

                def emit_rowsum():
                    # head hp's exps (and tree) finished a whole head ago
                    # for the interleaved heads, so the rowsum matmul +
                    # reciprocal + broadcast run as early as the PE reaches
                    # this section; the divide never gates the next head.
                    p = pm.tile([128, N], f32, tag=ptag, name=f"prs{hp}")
                    srcs = [uallb] if upt7 is None else [uallb, upt7]
                    for half in range(2):
                        for si, src in enumerate(srcs):
                            nc.tensor.matmul(
                                p[0:1, half * 512:(half + 1) * 512],
                                ones_r[:],
                                src[:, half * 512:(half + 1) * 512],
                                start=(si == 0), stop=(si == len(srcs) - 1))
                    # reciprocal + broadcast per half: each rowsum half's
                    # accumulation group stops independently, so half 0 of
                    # the divide chain (and the attn/proj work behind it)
                    # never waits for half 1
                    rr = ps.tile([1, N], f32, tag="rr", bufs=2,
                                 name=f"rr{hp}")
                    rb = ps.tile([128, N], f32, tag="rb", bufs=2,
                                 name=f"rb{hp}")
                    for hf in range(2):
                        sl = slice(hf * 512, (hf + 1) * 512)
                        nc.vector.reciprocal_approx_fast(rr[:, sl],
                                                         p[0:1, sl])
                        nc.gpsimd.partition_broadcast(rb[:, sl], rr[:, sl])
                    return rb

                rb = None
                if hp is not None and hp < NUM_HEADS - 1:
                    rb = emit_rowsum()
                for jt in range(NO_QK):
                    if hn is not None:
                        emit_st(hn, [jt])
                    if hp == NUM_HEADS - 1 and jt == NO_QK - 1:
                        # last head: its tree is only ready near the end, so
                        # the rowsum goes just before the final PV step
                        rb = emit_rowsum()
                    if hp is not None:
                        for half in range(2):
                            nc.tensor.matmul(
                                ppv[:, half * 512:(half + 1) * 512],
                                vs[jt][:, hp * 128:(hp + 1) * 128],
                                pts[jt][:, half * 512:(half + 1) * 512],
                                start=(jt == 0), stop=(jt == NO_QK - 1))
                if hp is None:
                    return
                # per-half multiplies: subtile dep tracking lets the proj
                # matmuls of each half start as soon as that half lands
                attn_h = pp.tile([128, N], bf16, tag=f"attn{hp}",
                                 name=f"attn{hp}")
                for hf in range(2):
                    sl = slice(hf * 512, (hf + 1) * 512)
                    nc.vector.tensor_mul(attn_h[:, sl], ppv[:, sl], rb[:, sl])
                attns[hp] = attn_h

            pprs = [None] * NT

            def emit_proj_mm(ot, h):
                for half in range(2):
                    nc.tensor.matmul(
                        pprs[ot][:, half * 512:(half + 1) * 512],
                        pwts[h][:, ot * 128:(ot + 1) * 128],
                        attns[h][:, half * 512:(half + 1) * 512],
                        start=(h == 0), stop=(h == NUM_HEADS - 1))

            def emit_out(ot):
                # even tiles: one DVE 3-input op. odd tiles: ACT does the
                # PSUM read + bias, DVE only the residual add — the pair of
                # output tiles of each proj wave then finishes in parallel
                # across the two engines.
                o_t = p_out.tile([128, N], bf16, tag="out", name=f"o{ot}")
                eng = nc.scalar if ot % 2 == 1 else nc.sync
                rows = slice(ot * 128, (ot + 1) * 128)
                if ot < 2:
                    if ot % 2 == 0:
                        nc.vector.scalar_tensor_tensor(o_t[:], pprs[ot][:],
                                                       beff[:, ot:ot + 1],
                                                       xbs[ot][:],
                                                       op0=Alu.add,
                                                       op1=Alu.add)
                    else:
                        ob = p_out.tile([128, N], bf16, tag="ob",
                                        name=f"ob{ot}")
                        nc.scalar.activation(ob[:], pprs[ot][:], Act.Identity,
                                             bias=beff[:, ot:ot + 1])
                        nc.vector.tensor_add(o_t[:], ob[:], xbs[ot][:])
                    eng.dma_start(d_out[rows, :], o_t[:])
                    return
                # last proj wave: store + DMA per half so the final bytes
                # leave ~1us earlier on the fully exposed tail
                ob = None
                if ot % 2 == 1:
                    ob = p_out.tile([128, N], bf16, tag="ob", name=f"ob{ot}")
                for hf in range(2):
                    sl = slice(hf * 512, (hf + 1) * 512)
                    if ot % 2 == 0:
                        nc.vector.scalar_tensor_tensor(o_t[:, sl],
                                                       pprs[ot][:, sl],
                                                       beff[:, ot:ot + 1],
                                                       xbs[ot][:, sl],
                                                       op0=Alu.add,
                                                       op1=Alu.add)
                    else:
                        nc.scalar.activation(ob[:, sl], pprs[ot][:, sl],
                                             Act.Identity,
                                             bias=beff[:, ot:ot + 1])
                        nc.vector.tensor_add(o_t[:, sl], ob[:, sl],
                                             xbs[ot][:, sl])
                    eng.dma_start(d_out[rows, sl], o_t[:, sl])

            # interleaved schedule: head 0's qkv came from wave A; remaining
            # qkv pairs + V feed heads as their dependencies resolve.
            emit_bias(0, pqq)
            emit_bias(4, pqk)
            emit_st(0)
            emit_qkv(1); emit_qkv(5)
            emit_st(1)
            emit_qkv(2); emit_qkv(6)
            emit_qkv(3); emit_qkv(7)
            for nt in range(NO_QK):
                emit_v(nt)
            u0, p7_0 = emit_tree(0)
            emit_pv_st(0, 2, u0, p7_0)
            u1, p7_1 = emit_tree(1)
            emit_pv_st(1, 3, u1, p7_1)
            u2, p7_2 = emit_tree(2)
            emit_pv_st(2, None, u2, p7_2)
            u3, p7_3 = emit_tree(3)
            emit_pv_st(3, None, u3, p7_3)
            # ---- proj in two PSUM waves; wave 1 accumulates heads as the
            # attn tiles land, wave 2 reuses the banks freed by the outputs
            # wave 1 rides the acc ring: head 2's tiles there are consumed
            # well before head 3's divide chain, so these proj matmuls can
            # accumulate heads 0-2 while attn3 is still being produced
            # (on the ps ring they would wait for attn3 via ppv3's slot)
            pprs[0] = pm.tile([128, N], f32, tag="acc", name="ppr0")
            pprs[1] = pm.tile([128, N], f32, tag="acc", name="ppr1")
            for h in range(NUM_HEADS):
                emit_proj_mm(0, h)
                emit_proj_mm(1, h)
            emit_out(0)
            emit_out(1)
            pprs[2] = pm.tile([128, N], f32, tag="ps", name="ppr2")
            pprs[3] = pm.tile([128, N], f32, tag="ps", name="ppr3")
            for h in range(NUM_HEADS):
                emit_proj_mm(2, h)
                emit_proj_mm(3, h)
            emit_out(2)
            emit_out(3)

    nc.compile()
    return nc


def make_in_maps(x, norm_w, norm_b, qkv_w, qkv_b, proj_w, proj_b):
    x = np.asarray(x, dtype=np.float32)
    qkv_w = np.asarray(qkv_w, dtype=np.float32)
    qkv_b = np.asarray(qkv_b, dtype=np.float32)
    proj_w = np.asarray(proj_w, dtype=np.float32)
    proj_b = np.asarray(proj_b, dtype=np.float32)

    import ml_dtypes
    wt = np.ascontiguousarray(qkv_w.T).astype(ml_dtypes.bfloat16)   # [C, 3C]
    # wave columns (q0|k0) loaded first; the rest follow
    wqkw = np.ascontiguousarray(
        np.concatenate([wt[:, 0:128], wt[:, 512:640]], axis=1))
    wrest = np.ascontiguousarray(
        np.concatenate([wt[:, 128:512], wt[:, 640:1024]], axis=1))
    wv = np.ascontiguousarray(wt[:, 2 * C:3 * C])
    pwt = np.ascontiguousarray(proj_w.T).astype(ml_dtypes.bfloat16)  # [C, C]
    b_eff = (proj_b + proj_w @ qkv_b[2 * C:3 * C]).astype(np.float32)
    bias_qk = np.ascontiguousarray(qkv_b[:2 * C])

    p = np.arange(128)
    sel = (p[:, None] // GROUP_CH == np.arange(8)[None, :]).astype(np.float32)
    inv_n = 1.0 / float(GROUP_CH * N)
    selsel = np.ascontiguousarray((sel @ sel.T) * inv_n)

    xs = x.reshape(B, C, N)
    cvec = np.zeros((128, 28), np.float32)
    cvec[:, 0:4] = np.asarray(norm_w, np.float32).reshape(4, 128).T
    cvec[:, 4:8] = np.asarray(norm_b, np.float32).reshape(4, 128).T
    cvec[:, 8:16] = bias_qk.reshape(8, 128).T
    cvec[:, 16:20] = b_eff.reshape(4, 128).T
    cvec[:, 20:28] = sel
    common = {
        "qkw_wt": wqkw, "qkrest_wt": wrest, "v_wt": wv, "proj_wt": pwt,
        "cvec": cvec, "selsel": selsel,
        "ones": np.ones((128, 1), ml_dtypes.bfloat16),
    }
    return [dict(common,
                 xb=np.ascontiguousarray(xs[i]).astype(ml_dtypes.bfloat16))
            for i in range(B)]


def run(inputs, trace=False, tmpdir=None):
    from concourse.bass_utils import run_bass_kernel_spmd
    nc = build_bass()
    in_maps = make_in_maps(**inputs)
    res = run_bass_kernel_spmd(nc, in_maps, core_ids=list(range(N_CORES)),
                               trace=trace, tmpdir=tmpdir)
    out = np.stack([res.results[i]["out"] for i in range(N_CORES)])
    return out.reshape(B, C, H, W).astype(np.float32), res


def kernel(**inputs):
    out, _ = run(inputs, trace=False)
    return out

